# revision 2
# baseline (speedup 1.0000x reference)
"""Distributed Trainium2 (Bass/Tile) kernel for a 16-head attention block.

Problem: x:[2,2048,1024], 16 heads of dim 64, full [B,H,S,S] additive bias,
softmax, out-projection.  Runs SPMD on 8 NeuronCores: mesh = batch(2) x
head-group(4), i.e. each core handles one batch element and 4 heads
(tensor-parallel split of wq/wk/wv columns and wo rows).  Each core emits a
partial [S, D] output; the host sums the 4 head-group partials per batch.

Device-side formulation (per core, heads h=0..3 local):
  QT/KT = (w @ x^T) in [dq, s] layout (transposed activations)
  S^T[k,q]  = K Q^T               (TensorE, contraction over head dim 64)
  P^T[k,q]  = exp(S^T/8) * exp(bias^T)[k,q]   (ScalarE exp, VectorE mult;
              exp(bias) is precomputed on the host and streamed as bf16 --
              exp(a+b) = exp(a)*exp(b) makes the bias add free)
  attnoutT/denoms via one matmul against V augmented with a ones column
  attnT = attnoutT * (1/denom) broadcast    (softmax normalization)
  out_partial = attnT^T @ woT               (TensorE)
All matmuls in bf16 with f32 PSUM accumulation; softmax math in f32.
"""

import os
import sys

try:
    import concourse  # noqa: F401
except ImportError:  # pragma: no cover - fallback for bare containers
    for _p in ("/opt/trn_rl_repo", os.path.expanduser("~/.axon_site/_ro/trn_rl_repo")):
        if os.path.isdir(_p) and _p not in sys.path:
            sys.path.insert(0, _p)

from contextlib import ExitStack

import ml_dtypes
import numpy as np

import concourse.mybir as mybir
import concourse.tile as tile
from concourse import bacc
from concourse.bass_utils import run_bass_kernel_spmd

BF16 = ml_dtypes.bfloat16

B, S, D = 2, 2048, 1024
H, HD = 16, 64
NCORES = 8
HG = 4                 # head groups (tensor-parallel factor)
HPG = H // HG          # heads per group = 4
DG = HPG * HD          # feature cols per group = 256
P = 128
KD = D // P            # contraction chunks for projections = 8
ST = S // P            # 128-row tiles along sequence = 16
NQB = S // 512         # 512-wide query blocks = 4

_CACHE = {}


def _build_nc():
    bf = mybir.dt.bfloat16
    f32 = mybir.dt.float32
    Exp = mybir.ActivationFunctionType.Exp
    mult = mybir.AluOpType.mult

    nc = bacc.Bacc("TRN2", target_bir_lowering=False, debug=False,
                   num_devices=NCORES)

    xT = nc.dram_tensor("xT", [D, S], bf, kind="ExternalInput")
    wqT = nc.dram_tensor("wqT", [D, DG], bf, kind="ExternalInput")
    wkT = nc.dram_tensor("wkT", [D, DG], bf, kind="ExternalInput")
    wvT = nc.dram_tensor("wvT", [D, DG], bf, kind="ExternalInput")
    woT = nc.dram_tensor("woT", [DG, D], bf, kind="ExternalInput")
    ebT = nc.dram_tensor("ebT", [HPG, S, S], bf, kind="ExternalInput")
    out = nc.dram_tensor("out", [S, D], f32, kind="ExternalOutput")

    with tile.TileContext(nc) as tc, ExitStack() as ctx:
        const = ctx.enter_context(tc.tile_pool(name="const", bufs=1))
        work = ctx.enter_context(tc.tile_pool(name="work", bufs=4))
        eb_pool = ctx.enter_context(tc.tile_pool(name="eb", bufs=8))
        pt_pool = ctx.enter_context(tc.tile_pool(name="pt", bufs=6))
        ev_pool = ctx.enter_context(tc.tile_pool(name="ev", bufs=3))
        outsb = ctx.enter_context(tc.tile_pool(name="outsb", bufs=4))
        psum_m = ctx.enter_context(tc.tile_pool(name="psum_m", bufs=2, space="PSUM"))
        psum_s = ctx.enter_context(tc.tile_pool(name="psum_s", bufs=2, space="PSUM"))
        psum_o = ctx.enter_context(tc.tile_pool(name="psum_o", bufs=4, space="PSUM"))

        # ---- persistent SBUF tensors ----
        xT_sb = const.tile([P, KD, S], bf, tag="xT_sb")
        wqT_sb = const.tile([P, KD, DG], bf, tag="wqT_sb")
        wkT_sb = const.tile([P, KD, DG], bf, tag="wkT_sb")
        wvT_sb = const.tile([P, KD, DG], bf, tag="wvT_sb")
        woT_sb = const.tile([P, DG // P, D], bf, tag="woT_sb")
        QT_sb = const.tile([P, DG // P, S], bf, tag="QT_sb")
        KT_sb = const.tile([P, DG // P, S], bf, tag="KT_sb")
        # V with a ones column appended per head: [s, h*(HD+1)]
        Vg_sb = const.tile([P, ST, HPG * (HD + 1)], bf, tag="Vg_sb")
        attnT_sb = const.tile([P, DG // P, S], bf, tag="attnT_sb")

        nc.sync.dma_start(out=xT_sb[:], in_=xT[:].rearrange("(o p) s -> p o s", p=P))
        nc.sync.dma_start(out=wqT_sb[:], in_=wqT[:].rearrange("(o p) c -> p o c", p=P))
        nc.sync.dma_start(out=wkT_sb[:], in_=wkT[:].rearrange("(o p) c -> p o c", p=P))
        nc.sync.dma_start(out=wvT_sb[:], in_=wvT[:].rearrange("(o p) c -> p o c", p=P))
        nc.sync.dma_start(out=woT_sb[:], in_=woT[:].rearrange("(o p) d -> p o d", p=P))

        # ones columns for V augmentation (V copies below overwrite the rest)
        nc.vector.memset(Vg_sb[:], 1.0)

        # ---- Q^T / K^T projections: [dq, s] = w @ x^T ----
        for w_sb, dst in ((wqT_sb, QT_sb), (wkT_sb, KT_sb)):
            for m in range(DG // P):
                for n in range(NQB):
                    ps = psum_m.tile([P, 512], f32, tag="m")
                    for kc in range(KD):
                        nc.tensor.matmul(
                            ps[:],
                            lhsT=w_sb[:, kc, m * P:(m + 1) * P],
                            rhs=xT_sb[:, kc, n * 512:(n + 1) * 512],
                            start=(kc == 0), stop=(kc == KD - 1),
                        )
                    nc.vector.tensor_copy(
                        out=dst[:, m, n * 512:(n + 1) * 512], in_=ps[:])

        # ---- V projection (natural [s, dv] layout) ----
        for t in range(ST):
            ps = psum_m.tile([P, DG], f32, tag="m")
            for kc in range(KD):
                nc.tensor.matmul(
                    ps[:],
                    lhsT=xT_sb[:, kc, t * P:(t + 1) * P],
                    rhs=wvT_sb[:, kc, :],
                    start=(kc == 0), stop=(kc == KD - 1),
                )
            for h in range(HPG):
                nc.vector.tensor_copy(
                    out=Vg_sb[:, t, h * (HD + 1):h * (HD + 1) + HD],
                    in_=ps[:, h * HD:(h + 1) * HD])

        # ---- attention: head pairs x query halves ----
        for hp in range(HPG // 2):
            for qbp in range(2):
                po = {}
                for h2 in range(2):
                    for qb2 in range(2):
                        po[h2, qb2] = psum_o.tile(
                            [P, 512], f32, tag="o", name=f"po_{h2}_{qb2}")
                for kt in range(ST):
                    for h2 in range(2):
                        h = hp * 2 + h2
                        ebt = eb_pool.tile([P, 1024], bf, tag="eb")
                        nc.sync.dma_start(
                            out=ebt[:],
                            in_=ebT[h, kt * P:(kt + 1) * P,
                                    qbp * 1024:(qbp + 1) * 1024])
                        for qb2 in range(2):
                            q0 = qbp * 1024 + qb2 * 512
                            ps = psum_s.tile([P, 512], f32, tag="s")
                            nc.tensor.matmul(
                                ps[:],
                                lhsT=KT_sb[h2 * HD:(h2 + 1) * HD, hp,
                                           kt * P:(kt + 1) * P],
                                rhs=QT_sb[h2 * HD:(h2 + 1) * HD, hp,
                                          q0:q0 + 512],
                                start=True, stop=True,
                            )
                            pt = pt_pool.tile([P, 512], bf, tag="pt")
                            # exp(S^T / sqrt(HD)) straight out of PSUM
                            nc.scalar.activation(pt[:], ps[:], Exp, scale=0.125)
                            nc.vector.tensor_tensor(
                                pt[:], pt[:],
                                ebt[:, qb2 * 512:(qb2 + 1) * 512], mult)
                            nc.tensor.matmul(
                                po[h2, qb2][:HD + 1, :],
                                lhsT=Vg_sb[:, kt, h * (HD + 1):(h + 1) * (HD + 1)],
                                rhs=pt[:],
                                start=(kt == 0), stop=(kt == ST - 1),
                            )
                # normalize and store to attnT
                for h2 in range(2):
                    for qb2 in range(2):
                        q0 = qbp * 1024 + qb2 * 512
                        acc = po[h2, qb2]
                        den = ev_pool.tile([1, 512], f32, tag="den")
                        nc.vector.tensor_copy(out=den[:], in_=acc[HD:HD + 1, :])
                        rc = ev_pool.tile([1, 512], f32, tag="rc")
                        nc.vector.reciprocal_approx_fast(out=rc[:], in_=den[:])
                        bc = ev_pool.tile([HD, 512], f32, tag="bc")
                        nc.gpsimd.partition_broadcast(bc[:], rc[:])
                        nc.vector.tensor_tensor(
                            attnT_sb[h2 * HD:(h2 + 1) * HD, hp, q0:q0 + 512],
                            acc[:HD, :], bc[:], mult)

        # ---- output projection: out[s, do] = attnT^T @ woT ----
        for st in range(ST):
            for nb in range(D // 512):
                ps = psum_m.tile([P, 512], f32, tag="m")
                for c in range(DG // P):
                    nc.tensor.matmul(
                        ps[:],
                        lhsT=attnT_sb[:, c, st * P:(st + 1) * P],
                        rhs=woT_sb[:, c, nb * 512:(nb + 1) * 512],
                        start=(c == 0), stop=(c == DG // P - 1),
                    )
                ob = outsb.tile([P, 512], f32, tag="ob")
                nc.vector.tensor_copy(out=ob[:], in_=ps[:])
                nc.sync.dma_start(
                    out=out[st * P:(st + 1) * P, nb * 512:(nb + 1) * 512],
                    in_=ob[:])

    nc.compile()
    return nc


def _get_nc():
    if "nc" not in _CACHE:
        _CACHE["nc"] = _build_nc()
    return _CACHE["nc"]


def kernel(x, mask, attn_bias, wq, wk, wv, wo):
    x = np.asarray(x, dtype=np.float32)
    mask = np.asarray(mask, dtype=np.float32)
    attn_bias = np.asarray(attn_bias, dtype=np.float32)
    wq = np.asarray(wq, dtype=np.float32)
    wk = np.asarray(wk, dtype=np.float32)
    wv = np.asarray(wv, dtype=np.float32)
    wo = np.asarray(wo, dtype=np.float32)

    bias = attn_bias
    if mask.any():
        bias = bias + mask  # broadcast [1,1,S,S] over [B,H,S,S]

    nc = _get_nc()

    in_maps = []
    for core in range(NCORES):
        b, hg = divmod(core, HG)
        c0, c1 = hg * DG, (hg + 1) * DG
        xTb = np.ascontiguousarray(x[b].T).astype(BF16)
        m = {
            "xT": xTb,
            "wqT": np.ascontiguousarray(wq[c0:c1, :].T).astype(BF16),
            "wkT": np.ascontiguousarray(wk[c0:c1, :].T).astype(BF16),
            "wvT": np.ascontiguousarray(wv[c0:c1, :].T).astype(BF16),
            "woT": np.ascontiguousarray(wo[:, c0:c1].T).astype(BF16),
            # exp(bias)^T per local head: [h, k, q]
            "ebT": np.ascontiguousarray(
                np.exp(bias[b, hg * HPG:(hg + 1) * HPG]).transpose(0, 2, 1)
            ).astype(BF16),
        }
        in_maps.append(m)

    res = run_bass_kernel_spmd(nc, in_maps, core_ids=list(range(NCORES)))

    full = np.zeros((B, S, D), dtype=np.float32)
    for core in range(NCORES):
        b = core // HG
        full[b] += np.asarray(res.results[core]["out"], dtype=np.float32)
    return full


# revision 3
# speedup vs baseline: 1.1788x; 1.1788x over previous
"""Distributed Trainium2 (Bass/Tile) kernel for a 16-head attention block.

Problem: x:[2,2048,1024], 16 heads of dim 64, full [B,H,S,S] additive bias,
softmax, out-projection.  Runs SPMD on 8 NeuronCores: mesh = batch(2) x
head-group(4), i.e. each core handles one batch element and 4 heads
(tensor-parallel split of wq/wk/wv columns and wo rows).  Each core emits a
partial [S, D] output; the host sums the 4 head-group partials per batch.

Device-side formulation (per core, heads h=0..3 local):
  QT/KT = (w @ x^T) in [dq, s] layout (transposed activations, Q pre-scaled
          by 1/sqrt(64))
  PSUM  = K Q^T  (TensorE, contraction over head dim 64)
        + I @ bias^T  (TensorE identity-matmul accumulate -- the bias add
          rides on the TensorE so the Scalar/Vector engines stay off the
          critical per-tile chain and the PE clock stays un-throttled)
  P^T   = exp(PSUM)                (one wide ScalarE op per [128,1024] tile)
  attnoutT/denoms via one matmul against V augmented with a ones column
  attnT = attnoutT * (1/denom) broadcast    (softmax normalization)
  out_partial = attnT^T @ woT               (TensorE)
All matmuls in bf16 with f32 PSUM accumulation; softmax math in f32.
"""

import os
import sys

try:
    import concourse  # noqa: F401
except ImportError:  # pragma: no cover - fallback for bare containers
    for _p in ("/opt/trn_rl_repo", os.path.expanduser("~/.axon_site/_ro/trn_rl_repo")):
        if os.path.isdir(_p) and _p not in sys.path:
            sys.path.insert(0, _p)

from contextlib import ExitStack

import ml_dtypes
import numpy as np

import concourse.mybir as mybir
import concourse.tile as tile
from concourse import bacc
from concourse.bass_utils import run_bass_kernel_spmd
from concourse.masks import make_identity

BF16 = ml_dtypes.bfloat16

B, S, D = 2, 2048, 1024
H, HD = 16, 64
NCORES = 8
HG = 4                 # head groups (tensor-parallel factor)
HPG = H // HG          # heads per group = 4
DG = HPG * HD          # feature cols per group = 256
P = 128
KD = D // P            # contraction chunks for projections = 8
ST = S // P            # 128-row tiles along sequence = 16
NQB = S // 512         # 512-wide query blocks = 4

_CACHE = {}


def _build_nc():
    bf = mybir.dt.bfloat16
    f32 = mybir.dt.float32
    Exp = mybir.ActivationFunctionType.Exp
    mult = mybir.AluOpType.mult

    nc = bacc.Bacc("TRN2", target_bir_lowering=False, debug=False,
                   num_devices=NCORES)

    xT = nc.dram_tensor("xT", [D, S], bf, kind="ExternalInput")
    wqT = nc.dram_tensor("wqT", [D, DG], bf, kind="ExternalInput")
    wkT = nc.dram_tensor("wkT", [D, DG], bf, kind="ExternalInput")
    wvT = nc.dram_tensor("wvT", [D, DG], bf, kind="ExternalInput")
    woT = nc.dram_tensor("woT", [DG, D], bf, kind="ExternalInput")
    ebT = nc.dram_tensor("ebT", [HPG, S, S], bf, kind="ExternalInput")
    out = nc.dram_tensor("out", [S, D], f32, kind="ExternalOutput")

    with tile.TileContext(nc) as tc, ExitStack() as ctx:
        const = ctx.enter_context(tc.tile_pool(name="const", bufs=1))
        eb_pool = ctx.enter_context(tc.tile_pool(name="eb", bufs=8))
        pt_pool = ctx.enter_context(tc.tile_pool(name="pt", bufs=6))
        ev_pool = ctx.enter_context(tc.tile_pool(name="ev", bufs=3))
        outsb = ctx.enter_context(tc.tile_pool(name="outsb", bufs=4))

        # ---- persistent SBUF tensors ----
        xT_sb = const.tile([P, KD, S], bf, tag="xT_sb")
        wqT_sb = const.tile([P, KD, DG], bf, tag="wqT_sb")
        wkT_sb = const.tile([P, KD, DG], bf, tag="wkT_sb")
        wvT_sb = const.tile([P, KD, DG], bf, tag="wvT_sb")
        woT_sb = const.tile([P, DG // P, D], bf, tag="woT_sb")
        QT_sb = const.tile([P, DG // P, S], bf, tag="QT_sb")
        KT_sb = const.tile([P, DG // P, S], bf, tag="KT_sb")
        # V with a ones column appended per head: [s, h*(HD+1)]
        Vg_sb = const.tile([P, ST, HPG * (HD + 1)], bf, tag="Vg_sb")
        attnT_sb = const.tile([P, DG // P, S], bf, tag="attnT_sb")
        ident_sb = const.tile([P, P], bf, tag="ident_sb")

        make_identity(nc, ident_sb[:])

        nc.sync.dma_start(out=xT_sb[:], in_=xT[:].rearrange("(o p) s -> p o s", p=P))
        nc.sync.dma_start(out=wqT_sb[:], in_=wqT[:].rearrange("(o p) c -> p o c", p=P))
        nc.sync.dma_start(out=wkT_sb[:], in_=wkT[:].rearrange("(o p) c -> p o c", p=P))
        nc.sync.dma_start(out=wvT_sb[:], in_=wvT[:].rearrange("(o p) c -> p o c", p=P))
        nc.sync.dma_start(out=woT_sb[:], in_=woT[:].rearrange("(o p) d -> p o d", p=P))

        # ones columns for V augmentation (V copies below overwrite the rest)
        nc.vector.memset(Vg_sb[:], 1.0)

        # ---- projections (own PSUM pool, closed before attention) ----
        with ExitStack() as ph1:
            psum_q = ph1.enter_context(
                tc.tile_pool(name="psum_q", bufs=2, space="PSUM"))
            for w_sb, dst, scale in ((wqT_sb, QT_sb, 0.125), (wkT_sb, KT_sb, None)):
                for m in range(DG // P):
                    for n in range(NQB):
                        ps = psum_q.tile([P, 512], f32, tag="q")
                        for kc in range(KD):
                            nc.tensor.matmul(
                                ps[:],
                                lhsT=w_sb[:, kc, m * P:(m + 1) * P],
                                rhs=xT_sb[:, kc, n * 512:(n + 1) * 512],
                                start=(kc == 0), stop=(kc == KD - 1),
                            )
                        dslice = dst[:, m, n * 512:(n + 1) * 512]
                        if scale is None:
                            nc.vector.tensor_copy(out=dslice, in_=ps[:])
                        else:
                            nc.vector.tensor_scalar_mul(dslice, ps[:], scale)

            for t in range(ST):
                ps = psum_q.tile([P, DG], f32, tag="q")
                for kc in range(KD):
                    nc.tensor.matmul(
                        ps[:],
                        lhsT=xT_sb[:, kc, t * P:(t + 1) * P],
                        rhs=wvT_sb[:, kc, :],
                        start=(kc == 0), stop=(kc == KD - 1),
                    )
                for h in range(HPG):
                    nc.vector.tensor_copy(
                        out=Vg_sb[:, t, h * (HD + 1):h * (HD + 1) + HD],
                        in_=ps[:, h * HD:(h + 1) * HD])

        # ---- attention: head pairs x query halves ----
        with ExitStack() as ph2:
            psum_s = ph2.enter_context(
                tc.tile_pool(name="psum_s", bufs=2, space="PSUM"))
            psum_o = ph2.enter_context(
                tc.tile_pool(name="psum_o", bufs=4, space="PSUM"))
            for hp in range(HPG // 2):
                for qbp in range(2):
                    q0 = qbp * 1024
                    po = {}
                    for h2 in range(2):
                        for qb2 in range(2):
                            po[h2, qb2] = psum_o.tile(
                                [P, 512], f32, tag="o", name=f"po_{h2}_{qb2}")
                    for kt in range(ST):
                        for h2 in range(2):
                            h = hp * 2 + h2
                            hs = slice(h2 * HD, (h2 + 1) * HD)
                            ebt = eb_pool.tile([P, 1024], bf, tag="eb")
                            nc.sync.dma_start(
                                out=ebt[:],
                                in_=ebT[h, kt * P:(kt + 1) * P, q0:q0 + 1024])
                            ps = psum_s.tile([P, 1024], f32, tag="s")
                            for qb2 in range(2):
                                nc.tensor.matmul(
                                    ps[:, qb2 * 512:(qb2 + 1) * 512],
                                    lhsT=KT_sb[hs, hp, kt * P:(kt + 1) * P],
                                    rhs=QT_sb[hs, hp,
                                              q0 + qb2 * 512:q0 + (qb2 + 1) * 512],
                                    start=True, stop=False,
                                )
                            for qb2 in range(2):
                                nc.tensor.matmul(
                                    ps[:, qb2 * 512:(qb2 + 1) * 512],
                                    lhsT=ident_sb[:],
                                    rhs=ebt[:, qb2 * 512:(qb2 + 1) * 512],
                                    start=False, stop=True,
                                )
                            pt = pt_pool.tile([P, 1024], bf, tag="pt")
                            nc.scalar.activation(pt[:], ps[:], Exp)
                            for qb2 in range(2):
                                nc.tensor.matmul(
                                    po[h2, qb2][:HD + 1, :],
                                    lhsT=Vg_sb[:, kt,
                                               h * (HD + 1):(h + 1) * (HD + 1)],
                                    rhs=pt[:, qb2 * 512:(qb2 + 1) * 512],
                                    start=(kt == 0), stop=(kt == ST - 1),
                                )
                    # normalize and store to attnT
                    for h2 in range(2):
                        for qb2 in range(2):
                            qq = q0 + qb2 * 512
                            acc = po[h2, qb2]
                            den = ev_pool.tile([1, 512], f32, tag="den")
                            nc.vector.tensor_copy(out=den[:], in_=acc[HD:HD + 1, :])
                            rc = ev_pool.tile([1, 512], f32, tag="rc")
                            nc.vector.reciprocal_approx_fast(out=rc[:], in_=den[:])
                            bc = ev_pool.tile([HD, 512], f32, tag="bc")
                            nc.gpsimd.partition_broadcast(bc[:], rc[:])
                            nc.vector.tensor_tensor(
                                attnT_sb[h2 * HD:(h2 + 1) * HD, hp, qq:qq + 512],
                                acc[:HD, :], bc[:], mult)

        # ---- output projection: out[s, do] = attnT^T @ woT ----
        with ExitStack() as ph3:
            psum_w = ph3.enter_context(
                tc.tile_pool(name="psum_w", bufs=2, space="PSUM"))
            for st in range(ST):
                for nb in range(D // 512):
                    ps = psum_w.tile([P, 512], f32, tag="w")
                    for c in range(DG // P):
                        nc.tensor.matmul(
                            ps[:],
                            lhsT=attnT_sb[:, c, st * P:(st + 1) * P],
                            rhs=woT_sb[:, c, nb * 512:(nb + 1) * 512],
                            start=(c == 0), stop=(c == DG // P - 1),
                        )
                    ob = outsb.tile([P, 512], f32, tag="ob")
                    nc.vector.tensor_copy(out=ob[:], in_=ps[:])
                    nc.sync.dma_start(
                        out=out[st * P:(st + 1) * P, nb * 512:(nb + 1) * 512],
                        in_=ob[:])

    nc.compile()
    return nc


def _get_nc():
    if "nc" not in _CACHE:
        _CACHE["nc"] = _build_nc()
    return _CACHE["nc"]


def kernel(x, mask, attn_bias, wq, wk, wv, wo):
    x = np.asarray(x, dtype=np.float32)
    mask = np.asarray(mask, dtype=np.float32)
    attn_bias = np.asarray(attn_bias, dtype=np.float32)
    wq = np.asarray(wq, dtype=np.float32)
    wk = np.asarray(wk, dtype=np.float32)
    wv = np.asarray(wv, dtype=np.float32)
    wo = np.asarray(wo, dtype=np.float32)

    bias = attn_bias
    if mask.any():
        bias = bias + mask  # broadcast [1,1,S,S] over [B,H,S,S]

    nc = _get_nc()

    in_maps = []
    for core in range(NCORES):
        b, hg = divmod(core, HG)
        c0, c1 = hg * DG, (hg + 1) * DG
        m = {
            "xT": np.ascontiguousarray(x[b].T).astype(BF16),
            "wqT": np.ascontiguousarray(wq[c0:c1, :].T).astype(BF16),
            "wkT": np.ascontiguousarray(wk[c0:c1, :].T).astype(BF16),
            "wvT": np.ascontiguousarray(wv[c0:c1, :].T).astype(BF16),
            "woT": np.ascontiguousarray(wo[:, c0:c1].T).astype(BF16),
            # bias^T per local head: [h, k, q]
            "ebT": np.ascontiguousarray(
                bias[b, hg * HPG:(hg + 1) * HPG].transpose(0, 2, 1)
            ).astype(BF16),
        }
        in_maps.append(m)

    res = run_bass_kernel_spmd(nc, in_maps, core_ids=list(range(NCORES)))

    full = np.zeros((B, S, D), dtype=np.float32)
    for core in range(NCORES):
        b = core // HG
        full[b] += np.asarray(res.results[core]["out"], dtype=np.float32)
    return full


# revision 5
# speedup vs baseline: 1.2129x; 1.0289x over previous
"""Distributed Trainium2 (Bass/Tile) kernel for a 16-head attention block.

Problem: x:[2,2048,1024], 16 heads of dim 64, full [B,H,S,S] additive bias,
softmax, out-projection.  Runs SPMD on 8 NeuronCores: mesh = batch(2) x
head-group(4), i.e. each core handles one batch element and 4 heads
(tensor-parallel split of wq/wk/wv columns and wo rows).  Each core emits a
partial [S, D] output; the host sums the 4 head-group partials per batch.

Device-side formulation (per core, heads h=0..3 local):
  QT/KT = (w @ x^T) in [dq, s] layout (transposed activations, Q pre-scaled
          by 1/sqrt(64))
  PSUM  = K Q^T  (TensorE, contraction over head dim 64)
        + I @ bias^T  (TensorE identity-matmul accumulate -- the bias add
          rides on the TensorE so the Scalar/Vector engines stay off the
          critical per-tile chain and the PE clock stays un-throttled)
  P^T   = exp(PSUM)                (one wide ScalarE op per [128,1024] tile)
  attnoutT/denoms via one matmul against V augmented with a ones column
  attnT = attnoutT * (1/denom) broadcast    (softmax normalization)
  out_partial = attnT^T @ woT               (TensorE)
All matmuls in bf16 with f32 PSUM accumulation; softmax math in f32.
"""

import os
import sys

try:
    import concourse  # noqa: F401
except ImportError:  # pragma: no cover - fallback for bare containers
    for _p in ("/opt/trn_rl_repo", os.path.expanduser("~/.axon_site/_ro/trn_rl_repo")):
        if os.path.isdir(_p) and _p not in sys.path:
            sys.path.insert(0, _p)

from contextlib import ExitStack

import ml_dtypes
import numpy as np

import concourse.mybir as mybir
import concourse.tile as tile
from concourse import bacc
from concourse.bass_utils import run_bass_kernel_spmd
from concourse.masks import make_identity

BF16 = ml_dtypes.bfloat16

B, S, D = 2, 2048, 1024
H, HD = 16, 64
NCORES = 8
HG = 4                 # head groups (tensor-parallel factor)
HPG = H // HG          # heads per group = 4
DG = HPG * HD          # feature cols per group = 256
P = 128
KD = D // P            # contraction chunks for projections = 8
ST = S // P            # 128-row tiles along sequence = 16
NQB = S // 512         # 512-wide query blocks = 4

_CACHE = {}


def _build_nc():
    bf = mybir.dt.bfloat16
    f32 = mybir.dt.float32
    Exp = mybir.ActivationFunctionType.Exp
    mult = mybir.AluOpType.mult

    nc = bacc.Bacc("TRN2", target_bir_lowering=False, debug=False,
                   num_devices=NCORES)

    xT = nc.dram_tensor("xT", [D, S], bf, kind="ExternalInput")
    wqT = nc.dram_tensor("wqT", [D, DG], bf, kind="ExternalInput")
    wkT = nc.dram_tensor("wkT", [D, DG], bf, kind="ExternalInput")
    wvT = nc.dram_tensor("wvT", [D, DG], bf, kind="ExternalInput")
    woT = nc.dram_tensor("woT", [DG, D], bf, kind="ExternalInput")
    ebT = nc.dram_tensor("ebT", [HPG, S, S], bf, kind="ExternalInput")
    out = nc.dram_tensor("out", [S, D], f32, kind="ExternalOutput")

    with tile.TileContext(nc) as tc, ExitStack() as ctx:
        const = ctx.enter_context(tc.tile_pool(name="const", bufs=1))
        eb_pool = ctx.enter_context(tc.tile_pool(name="eb", bufs=8))
        pt_pool = ctx.enter_context(tc.tile_pool(name="pt", bufs=6))
        ev_pool = ctx.enter_context(tc.tile_pool(name="ev", bufs=3))
        outsb = ctx.enter_context(tc.tile_pool(name="outsb", bufs=4))

        # ---- persistent SBUF tensors ----
        xT_sb = const.tile([P, KD, S], bf, tag="xT_sb")
        wqT_sb = const.tile([P, KD, DG], bf, tag="wqT_sb")
        wkT_sb = const.tile([P, KD, DG], bf, tag="wkT_sb")
        wvT_sb = const.tile([P, KD, DG], bf, tag="wvT_sb")
        woT_sb = const.tile([P, DG // P, D], bf, tag="woT_sb")
        QT_sb = const.tile([P, DG // P, S], bf, tag="QT_sb")
        KT_sb = const.tile([P, DG // P, S], bf, tag="KT_sb")
        # V with a ones column appended per head: [s, h*(HD+1)]
        Vg_sb = const.tile([P, ST, HPG * (HD + 1)], bf, tag="Vg_sb")
        attnT_sb = const.tile([P, DG // P, S], bf, tag="attnT_sb")
        ident_sb = const.tile([P, P], bf, tag="ident_sb")

        make_identity(nc, ident_sb[:])

        nc.sync.dma_start(out=xT_sb[:], in_=xT[:].rearrange("(o p) s -> p o s", p=P))
        nc.sync.dma_start(out=wqT_sb[:], in_=wqT[:].rearrange("(o p) c -> p o c", p=P))
        nc.sync.dma_start(out=wkT_sb[:], in_=wkT[:].rearrange("(o p) c -> p o c", p=P))
        nc.sync.dma_start(out=wvT_sb[:], in_=wvT[:].rearrange("(o p) c -> p o c", p=P))
        nc.sync.dma_start(out=woT_sb[:], in_=woT[:].rearrange("(o p) d -> p o d", p=P))

        # ones columns for V augmentation (V copies below overwrite the rest)
        nc.vector.memset(Vg_sb[:], 1.0)

        # ---- projections (own PSUM pool, closed before attention) ----
        with ExitStack() as ph1:
            psum_q = ph1.enter_context(
                tc.tile_pool(name="psum_q", bufs=2, space="PSUM"))
            for w_sb, dst, scale in ((wqT_sb, QT_sb, 0.125), (wkT_sb, KT_sb, None)):
                for m in range(DG // P):
                    for n in range(NQB):
                        ps = psum_q.tile([P, 512], f32, tag="q")
                        for kc in range(KD):
                            nc.tensor.matmul(
                                ps[:],
                                lhsT=w_sb[:, kc, m * P:(m + 1) * P],
                                rhs=xT_sb[:, kc, n * 512:(n + 1) * 512],
                                start=(kc == 0), stop=(kc == KD - 1),
                            )
                        dslice = dst[:, m, n * 512:(n + 1) * 512]
                        if scale is None:
                            nc.vector.tensor_copy(out=dslice, in_=ps[:])
                        else:
                            nc.vector.tensor_scalar_mul(dslice, ps[:], scale)

            for t in range(ST):
                ps = psum_q.tile([P, DG], f32, tag="q")
                for kc in range(KD):
                    nc.tensor.matmul(
                        ps[:],
                        lhsT=xT_sb[:, kc, t * P:(t + 1) * P],
                        rhs=wvT_sb[:, kc, :],
                        start=(kc == 0), stop=(kc == KD - 1),
                    )
                for h in range(HPG):
                    nc.vector.tensor_copy(
                        out=Vg_sb[:, t, h * (HD + 1):h * (HD + 1) + HD],
                        in_=ps[:, h * HD:(h + 1) * HD])

        # ---- attention: one head x query half per pass ----
        with ExitStack() as ph2:
            psum_s = ph2.enter_context(
                tc.tile_pool(name="psum_s", bufs=3, space="PSUM"))
            psum_o = ph2.enter_context(
                tc.tile_pool(name="psum_o", bufs=2, space="PSUM"))
            for h in range(HPG):
                hp, h2 = divmod(h, 2)
                hs = slice(h2 * HD, (h2 + 1) * HD)
                for qbp in range(2):
                    q0 = qbp * 1024
                    po = {}
                    for qb2 in range(2):
                        po[qb2] = psum_o.tile(
                            [P, 512], f32, tag="o", name=f"po_{qb2}")
                    for kt in range(ST):
                        ebt = eb_pool.tile([P, 1024], bf, tag="eb")
                        # spread DMA descriptor issue across two engines
                        dma_eng = nc.sync if kt % 2 == 0 else nc.gpsimd
                        dma_eng.dma_start(
                            out=ebt[:],
                            in_=ebT[h, kt * P:(kt + 1) * P, q0:q0 + 1024])
                        ps = psum_s.tile([P, 1024], f32, tag="s")
                        # bias first (start=True), scores accumulate on top:
                        # keeps the identity stationary loaded across both mms
                        for qb2 in range(2):
                            nc.tensor.matmul(
                                ps[:, qb2 * 512:(qb2 + 1) * 512],
                                lhsT=ident_sb[:],
                                rhs=ebt[:, qb2 * 512:(qb2 + 1) * 512],
                                start=True, stop=False,
                            )
                        for qb2 in range(2):
                            nc.tensor.matmul(
                                ps[:, qb2 * 512:(qb2 + 1) * 512],
                                lhsT=KT_sb[hs, hp, kt * P:(kt + 1) * P],
                                rhs=QT_sb[hs, hp,
                                          q0 + qb2 * 512:q0 + (qb2 + 1) * 512],
                                start=False, stop=True,
                            )
                        pt = pt_pool.tile([P, 1024], bf, tag="pt")
                        nc.scalar.activation(pt[:], ps[:], Exp)
                        for qb2 in range(2):
                            nc.tensor.matmul(
                                po[qb2][:HD + 1, :],
                                lhsT=Vg_sb[:, kt,
                                           h * (HD + 1):(h + 1) * (HD + 1)],
                                rhs=pt[:, qb2 * 512:(qb2 + 1) * 512],
                                start=(kt == 0), stop=(kt == ST - 1),
                            )
                    # normalize and store to attnT
                    for qb2 in range(2):
                        qq = q0 + qb2 * 512
                        acc = po[qb2]
                        den = ev_pool.tile([1, 512], f32, tag="den")
                        nc.vector.tensor_copy(out=den[:], in_=acc[HD:HD + 1, :])
                        rc = ev_pool.tile([1, 512], f32, tag="rc")
                        nc.vector.reciprocal_approx_fast(out=rc[:], in_=den[:])
                        bc = ev_pool.tile([HD, 512], f32, tag="bc")
                        nc.gpsimd.partition_broadcast(bc[:], rc[:])
                        nc.vector.tensor_tensor(
                            attnT_sb[h2 * HD:(h2 + 1) * HD, hp, qq:qq + 512],
                            acc[:HD, :], bc[:], mult)

        # ---- output projection: out[s, do] = attnT^T @ woT ----
        with ExitStack() as ph3:
            psum_w = ph3.enter_context(
                tc.tile_pool(name="psum_w", bufs=2, space="PSUM"))
            for st in range(ST):
                ob = outsb.tile([P, D], f32, tag="ob")
                for nb in range(D // 512):
                    ps = psum_w.tile([P, 512], f32, tag="w")
                    for c in range(DG // P):
                        nc.tensor.matmul(
                            ps[:],
                            lhsT=attnT_sb[:, c, st * P:(st + 1) * P],
                            rhs=woT_sb[:, c, nb * 512:(nb + 1) * 512],
                            start=(c == 0), stop=(c == DG // P - 1),
                        )
                    nc.vector.tensor_copy(
                        out=ob[:, nb * 512:(nb + 1) * 512], in_=ps[:])
                nc.gpsimd.dma_start(out=out[st * P:(st + 1) * P, :], in_=ob[:])

    nc.compile()
    return nc


def _get_nc():
    if "nc" not in _CACHE:
        _CACHE["nc"] = _build_nc()
    return _CACHE["nc"]


def kernel(x, mask, attn_bias, wq, wk, wv, wo):
    x = np.asarray(x, dtype=np.float32)
    mask = np.asarray(mask, dtype=np.float32)
    attn_bias = np.asarray(attn_bias, dtype=np.float32)
    wq = np.asarray(wq, dtype=np.float32)
    wk = np.asarray(wk, dtype=np.float32)
    wv = np.asarray(wv, dtype=np.float32)
    wo = np.asarray(wo, dtype=np.float32)

    bias = attn_bias
    if mask.any():
        bias = bias + mask  # broadcast [1,1,S,S] over [B,H,S,S]

    nc = _get_nc()

    in_maps = []
    for core in range(NCORES):
        b, hg = divmod(core, HG)
        c0, c1 = hg * DG, (hg + 1) * DG
        m = {
            "xT": np.ascontiguousarray(x[b].T).astype(BF16),
            "wqT": np.ascontiguousarray(wq[c0:c1, :].T).astype(BF16),
            "wkT": np.ascontiguousarray(wk[c0:c1, :].T).astype(BF16),
            "wvT": np.ascontiguousarray(wv[c0:c1, :].T).astype(BF16),
            "woT": np.ascontiguousarray(wo[:, c0:c1].T).astype(BF16),
            # bias^T per local head: [h, k, q]
            "ebT": np.ascontiguousarray(
                bias[b, hg * HPG:(hg + 1) * HPG].transpose(0, 2, 1)
            ).astype(BF16),
        }
        in_maps.append(m)

    res = run_bass_kernel_spmd(nc, in_maps, core_ids=list(range(NCORES)))

    full = np.zeros((B, S, D), dtype=np.float32)
    for core in range(NCORES):
        b = core // HG
        full[b] += np.asarray(res.results[core]["out"], dtype=np.float32)
    return full
